# revision 30
# baseline (speedup 1.0000x reference)
"""Distributed Bass kernel for nn_ArchetipesNetwork (sequential archetype RNN).

Sharding: modules axis M=64 split across 8 NeuronCores (8 modules/core).
Per-core weights live in SBUF; the time loop is fully unrolled on device.
Each step all-gathers the (M,H) module outputs (hy) for the interconnection
feedback C @ (outs @ Wm.T); the AllGather overlaps the local Wh matvecs.

Canonical on-chip layout "L" for all per-module (ML,H) state/params:
  SBUF tile [128, OC*ML]; tile[p, oc*ML + m] = V[m, oc*128 + p]
This keeps the hidden axis's low 7 bits on partitions for both the matvec
rhs (contraction over o) and matmul outputs (partition over h), so no
on-device transposes are ever needed.
"""

import numpy as np

T, I, M, H = 256, 128, 64, 512
DT = 0.01
NCORES = 8
ML = M // NCORES      # 8 local modules
OC = H // 128         # 4 chunks of the hidden axis
NCOLS = OC * ML       # 32 canonical columns

# matmul input dtype: "float32" or "bfloat16"
MM_DTYPE = "bfloat16"


def _to_L(v):
    """(ML, H) -> canonical [128, OC*ML] layout."""
    v = np.asarray(v, np.float32)
    return np.ascontiguousarray(
        v.reshape(ML, OC, 128).transpose(2, 1, 0)
    ).reshape(128, NCOLS)


def _prep_core_inputs(c, x, initial_states, Wm, C, Wx, Wh, bias, gamma, eps, mm_np):
    m0 = c * ML
    Wh_l = np.asarray(Wh[m0:m0 + ML], np.float32)    # (ML, H, H) [m, h, o]
    Wx_l = np.asarray(Wx[m0:m0 + ML], np.float32)    # (ML, H, I)
    C_l = np.asarray(C[m0:m0 + ML], np.float32)      # (ML, M)
    ini = np.asarray(initial_states[m0:m0 + ML], np.float32)

    whT = Wh_l.reshape(ML, H, OC, 128).transpose(3, 0, 2, 1)  # [p, m, oc, h]
    whT = np.ascontiguousarray(whT).reshape(128, ML * OC * H).astype(mm_np)
    wmT = np.asarray(Wm, np.float32).reshape(H, OC, 128).transpose(2, 1, 0)
    wmT = np.ascontiguousarray(wmT).reshape(128, OC * H).astype(mm_np)
    cT = np.zeros((128, ML), mm_np)
    cT[:M, :] = C_l.T.astype(mm_np)
    xT = np.ascontiguousarray(np.asarray(x, np.float32).T)       # [I, T]
    wxT = np.ascontiguousarray(Wx_l.transpose(2, 0, 1)).reshape(128, ML * H)

    return {
        "whT": whT,
        "wmT": wmT,
        "cT": cT,
        "xT": xT,
        "wxT": wxT,
        "biasL": _to_L(bias[m0:m0 + ML]),
        "aL": _to_L(1.0 - DT * np.asarray(eps[m0:m0 + ML], np.float32)),
        "g2L": _to_L(DT * np.asarray(gamma[m0:m0 + ML], np.float32)),
        "hy0L": _to_L(ini[:, 0]),
        "hz0L": _to_L(ini[:, 1]),
    }


def build(nc, t_steps=T, mm_dtype=MM_DTYPE):
    import concourse.mybir as mybir
    import concourse.tile as tile
    from concourse.bass import ts
    from contextlib import ExitStack

    F32 = mybir.dt.float32
    MMDT = getattr(mybir.dt, mm_dtype)
    AF = mybir.ActivationFunctionType
    ALU = mybir.AluOpType

    d_whT = nc.dram_tensor("whT", [128, ML * OC * H], MMDT, kind="ExternalInput").ap()
    d_wmT = nc.dram_tensor("wmT", [128, OC * H], MMDT, kind="ExternalInput").ap()
    d_cT = nc.dram_tensor("cT", [128, ML], MMDT, kind="ExternalInput").ap()
    d_xT = nc.dram_tensor("xT", [128, T], F32, kind="ExternalInput").ap()
    d_wxT = nc.dram_tensor("wxT", [128, ML * H], F32, kind="ExternalInput").ap()
    d_biasL = nc.dram_tensor("biasL", [128, NCOLS], F32, kind="ExternalInput").ap()
    d_aL = nc.dram_tensor("aL", [128, NCOLS], F32, kind="ExternalInput").ap()
    d_g2L = nc.dram_tensor("g2L", [128, NCOLS], F32, kind="ExternalInput").ap()
    d_hy0 = nc.dram_tensor("hy0L", [128, NCOLS], F32, kind="ExternalInput").ap()
    d_hz0 = nc.dram_tensor("hz0L", [128, NCOLS], F32, kind="ExternalInput").ap()

    # outputs stay in canonical layout on device; host transposes
    d_states = nc.dram_tensor(
        "out_states", [t_steps, 2, 128, NCOLS], F32, kind="ExternalOutput"
    ).ap()
    d_fb = nc.dram_tensor(
        "out_fb", [t_steps, 128, NCOLS], F32, kind="ExternalOutput"
    ).ap()

    with tile.TileContext(nc) as tc, ExitStack() as ctx:
        const = ctx.enter_context(tc.tile_pool(name="const", bufs=1))

        whT = const.tile([128, ML * OC * H], MMDT)
        # per-(m,oc) chunk loads so each consumer matmul waits on one queue
        for mo in range(ML * OC):
            nc.sync.dma_start(whT[:, ts(mo, H)], d_whT[:, ts(mo, H)])
        wmT = const.tile([128, OC * H], MMDT)
        for oc in range(OC):
            nc.sync.dma_start(wmT[:, ts(oc, H)], d_wmT[:, ts(oc, H)])
        cT = const.tile([128, ML], MMDT)
        nc.sync.dma_start(cT[:], d_cT)
        biasL = const.tile([128, NCOLS], F32)
        nc.sync.dma_start(biasL[:], d_biasL)
        aL = const.tile([128, NCOLS], F32)
        nc.sync.dma_start(aL[:], d_aL)
        g2L = const.tile([128, NCOLS], F32)
        nc.sync.dma_start(g2L[:], d_g2L)
        pxb = const.tile([128, t_steps * NCOLS], F32)

        # ---- px precompute: pxb[:, t*NCOLS + hc*ML+m] = (Wx[m] @ x_t)[h] + bias ----
        with tc.tile_pool(name="pxio", bufs=1) as pxio, \
             tc.tile_pool(name="pxps", bufs=2, space="PSUM") as pxps:
            xT = pxio.tile([128, T], F32)
            nc.sync.dma_start(xT[:], d_xT)
            wxT = pxio.tile([128, ML * H], F32)
            for m in range(ML):
                nc.sync.dma_start(wxT[:, ts(m, H)], d_wxT[:, ts(m, H)])
            pxb3 = pxb[:].rearrange("p (t c) -> p t c", c=NCOLS)
            for m in range(ML):
                for hc in range(OC):
                    ps = pxps.tile([128, t_steps], F32, tag="pxps")
                    nc.tensor.matmul(
                        ps[:],
                        lhsT=wxT[:, m * H + hc * 128: m * H + hc * 128 + 128],
                        rhs=xT[:, 0:t_steps],
                        start=True, stop=True,
                    )
                    nc.vector.tensor_scalar_add(
                        pxb3[:, :, hc * ML + m], ps[:],
                        biasL[:, hc * ML + m: hc * ML + m + 1],
                    )

        psum_pre = ctx.enter_context(tc.tile_pool(name="prep", bufs=3, space="PSUM"))
        psum_fb = ctx.enter_context(tc.tile_pool(name="fbp", bufs=2, space="PSUM"))
        psum_yt = ctx.enter_context(tc.tile_pool(name="ytp", bufs=2, space="PSUM"))
        sb = ctx.enter_context(tc.tile_pool(name="sb", bufs=4))
        state = ctx.enter_context(tc.tile_pool(name="state", bufs=4))
        dram = ctx.enter_context(tc.tile_pool(name="dram", bufs=4, space="DRAM"))
        # collective outputs must be addr_space=Shared for the fast path
        ag_outs = [
            nc.dram_tensor(f"ag_out_{i}", [NCORES * NCOLS, 128], MMDT,
                           addr_space="Shared")
            for i in range(4)
        ]

        outs_sb = []
        for i in range(2):
            o = const.tile([128, H], MMDT, tag=f"outs{i}")
            nc.vector.memset(o[M:128, :], 0.0)
            outs_sb.append(o)

        hyL = state.tile([128, NCOLS], F32, tag="hy")
        nc.sync.dma_start(hyL[:], d_hy0)
        hzL = state.tile([128, NCOLS], F32, tag="hz")
        nc.sync.dma_start(hzL[:], d_hz0)

        cast = MMDT != F32

        for t in range(t_steps):
            # ---- AllGather hy_t (outs_0 is zeros, so skip at t=0) ----
            if t > 0:
                src = src_prev
                hyT = sb.tile([NCOLS, 128], MMDT, tag="hyT")
                for bp in range(128 // 32):
                    nc.vector.transpose(
                        hyT[:, ts(bp, 32)], src[ts(bp, 32), :])
                ag_in = dram.tile([NCOLS, 128], MMDT, tag="agin")
                ag_out = ag_outs[t % 4]
                nc.sync.dma_start(ag_in[:], hyT[:])
                nc.gpsimd.collective_compute(
                    "AllGather", mybir.AluOpType.bypass,
                    replica_groups=[list(range(NCORES))],
                    ins=[ag_in[:].opt()], outs=[ag_out.ap().opt()],
                )
                outs_t = outs_sb[t % 2]
                # ag_out rows are (rank, m, oc): two half-loads on separate
                # queues; each covers 2 oc-chunks so PE wakes at most twice
                ag3 = ag_out.ap().rearrange(
                    "(mg q) p -> mg q p", q=OC)
                for half in range(2):
                    eng = nc.sync if half == 0 else nc.scalar
                    o0 = half * 2
                    eng.dma_start(
                        outs_t[0:M, o0 * 128:(o0 + 2) * 128]
                        .rearrange("mg (q p) -> mg q p", p=128),
                        ag3[:, o0:o0 + 2])

            # ---- pre = Wh @ hy (psum) ; px+bias+fb added later on DVE ----
            pre_ps = psum_pre.tile([128, NCOLS], F32, tag="pre")
            # Wh matmul rhs columns: at t=0 build the cast tile from hy0
            if t == 0:
                src = sb.tile([128, NCOLS], MMDT, tag="hycast")
                nc.vector.tensor_copy(
                    src[:].rearrange("p (m oc) -> p oc m", oc=OC),
                    hyL[:].rearrange("p (oc m) -> p oc m", m=ML))
            for m in range(ML):
                for hc in range(OC):
                    for oc in range(OC):
                        base = (m * OC + oc) * H + hc * 128
                        nc.tensor.matmul(
                            pre_ps[:, hc * ML + m: hc * ML + m + 1],
                            lhsT=whT[:, base: base + 128],
                            rhs=src[:, m * OC + oc: m * OC + oc + 1],
                            start=(oc == 0), stop=(oc == OC - 1),
                        )

            preX = sb.tile([128, NCOLS], F32, tag="preX")
            nc.vector.tensor_add(preX[:], pre_ps[:], pxb[:, ts(t, NCOLS)])

            # ---- feedback fb = (C_local @ outs) @ Wm.T ----
            fb_sb = sb.tile([128, NCOLS], F32, tag="fbs")
            if t == 0:
                nc.vector.memset(fb_sb[:], 0.0)
            else:
                yt_ps = psum_yt.tile([128, NCOLS], F32, tag="yt")
                yt_sb = sb.tile([128, NCOLS], MMDT, tag="yts")
                for oc in range(OC):
                    nc.tensor.matmul(
                        yt_ps[:, ts(oc, ML)],
                        lhsT=outs_t[:, ts(oc, 128)], rhs=cT[:],
                        start=True, stop=True,
                    )
                nc.vector.tensor_copy(yt_sb[:], yt_ps[:])
                fb_ps = psum_fb.tile([128, NCOLS], F32, tag="fb")
                for hc in range(OC):
                    for oc in range(OC):
                        nc.tensor.matmul(
                            fb_ps[:, ts(hc, ML)],
                            lhsT=wmT[:, oc * H + hc * 128: oc * H + hc * 128 + 128],
                            rhs=yt_sb[:, ts(oc, ML)],
                            start=(oc == 0), stop=(oc == OC - 1),
                        )

            # tanh input = (Wh@hy + px + bias) [precomputed in preX] + fb
            th_in = sb.tile([128, NCOLS], F32, tag="thin")
            if t == 0:
                nc.vector.tensor_copy(th_in[:], preX[:])
            else:
                nc.vector.tensor_add(th_in[:], fb_ps[:], preX[:])
            th = sb.tile([128, NCOLS], F32, tag="th")
            nc.scalar.activation(th[:], th_in[:], AF.Tanh)
            if t > 0:
                nc.scalar.copy(fb_sb[:], fb_ps[:])
            nc.gpsimd.dma_start(d_fb[t], fb_sb[:])

            # ---- hz' = a*hz - (DT*g)*hy + DT*tanh ; hy' = hy + DT*hz' ----
            # hz' = t3 + DT*th ; hy' = hy + DT*hz' = (hy + DT*t3) + DT^2*th
            hyn = state.tile([128, NCOLS], F32, tag="hy")
            hzn = state.tile([128, NCOLS], F32, tag="hz")
            tmp = sb.tile([128, NCOLS], F32, tag="tmp")
            tmp2 = sb.tile([128, NCOLS], F32, tag="tmp2")
            tmp4 = sb.tile([128, NCOLS], F32, tag="tmp4")
            nc.vector.tensor_mul(tmp[:], hzL[:], aL[:])
            nc.vector.tensor_mul(tmp2[:], hyL[:], g2L[:])
            nc.vector.tensor_sub(tmp[:], tmp[:], tmp2[:])
            nc.vector.scalar_tensor_tensor(
                tmp4[:], tmp[:], DT, hyL[:], op0=ALU.mult, op1=ALU.add)
            src_next = sb.tile([128, NCOLS], MMDT, tag="hycast")
            nc.vector.scalar_tensor_tensor(
                src_next[:].rearrange("p (m oc) -> p oc m", oc=OC),
                th[:].rearrange("p (oc m) -> p oc m", m=ML),
                DT * DT,
                tmp4[:].rearrange("p (oc m) -> p oc m", m=ML),
                op0=ALU.mult, op1=ALU.add)
            nc.vector.scalar_tensor_tensor(
                hyn[:], th[:], DT * DT, tmp4[:], op0=ALU.mult, op1=ALU.add)

            nc.vector.scalar_tensor_tensor(
                hzn[:], th[:], DT, tmp[:], op0=ALU.mult, op1=ALU.add)
            nc.gpsimd.dma_start(d_states[t, 0], hyn[:])
            nc.gpsimd.dma_start(d_states[t, 1], hzn[:])
            hyL, hzL = hyn, hzn
            src_prev = src_next

    return nc


_CACHE = {}


def _get_nc(t_steps=T, mm_dtype=MM_DTYPE):
    key = (t_steps, mm_dtype)
    if key not in _CACHE:
        import concourse.bacc as bacc
        nc = bacc.Bacc("TRN2", target_bir_lowering=False, debug=False,
                       num_devices=NCORES)
        build(nc, t_steps, mm_dtype)
        nc.compile()
        _CACHE[key] = nc
    return _CACHE[key]


def run(t_steps=T, mm_dtype=MM_DTYPE, trace=False, **inputs):
    import ml_dtypes
    from concourse.bass_utils import run_bass_kernel_spmd

    mm_np = np.float32 if mm_dtype == "float32" else ml_dtypes.bfloat16
    nc = _get_nc(t_steps, mm_dtype)
    in_maps = [
        _prep_core_inputs(c, mm_np=mm_np, **inputs) for c in range(NCORES)
    ]
    res = run_bass_kernel_spmd(
        nc, in_maps, core_ids=list(range(NCORES)), trace=trace,
    )

    initial_states = np.asarray(inputs["initial_states"], np.float32)
    states_all = np.empty((t_steps + 1, M, 2, H), np.float32)
    states_all[0] = initial_states
    seq_fb = np.empty((t_steps, M, H), np.float32)
    for c in range(NCORES):
        r = res.results[c]
        # out_states [t, 2, 128, NCOLS] canonical -> (t, ML, 2, H)
        st = r["out_states"].reshape(t_steps, 2, 128, OC, ML)
        states_all[1:, c * ML:(c + 1) * ML] = st.transpose(0, 4, 1, 3, 2).reshape(
            t_steps, ML, 2, H)
        fb = r["out_fb"].reshape(t_steps, 128, OC, ML)
        seq_fb[:, c * ML:(c + 1) * ML] = fb.transpose(0, 3, 2, 1).reshape(
            t_steps, ML, H)
    return (states_all, seq_fb), res


def kernel(**inputs):
    (states_all, seq_fb), _ = run(**inputs)
    return states_all, seq_fb


# revision 31
# speedup vs baseline: 1.0589x; 1.0589x over previous
"""Distributed Bass kernel for nn_ArchetipesNetwork (sequential archetype RNN).

Sharding: modules axis M=64 split across 8 NeuronCores (8 modules/core).
Per-core weights live in SBUF; the time loop is fully unrolled on device.
Each step all-gathers the (M,H) module outputs (hy) for the interconnection
feedback C @ (outs @ Wm.T); the AllGather overlaps the local Wh matvecs.

Canonical on-chip layout "L" for all per-module (ML,H) state/params:
  SBUF tile [128, OC*ML]; tile[p, oc*ML + m] = V[m, oc*128 + p]
This keeps the hidden axis's low 7 bits on partitions for both the matvec
rhs (contraction over o) and matmul outputs (partition over h), so no
on-device transposes are ever needed.
"""

import numpy as np

T, I, M, H = 256, 128, 64, 512
DT = 0.01
NCORES = 8
ML = M // NCORES      # 8 local modules
OC = H // 128         # 4 chunks of the hidden axis
NCOLS = OC * ML       # 32 canonical columns

# matmul input dtype: "float32" or "bfloat16"
MM_DTYPE = "bfloat16"


def _to_L(v):
    """(ML, H) -> canonical [128, OC*ML] layout."""
    v = np.asarray(v, np.float32)
    return np.ascontiguousarray(
        v.reshape(ML, OC, 128).transpose(2, 1, 0)
    ).reshape(128, NCOLS)


def _prep_core_inputs(c, x, initial_states, Wm, C, Wx, Wh, bias, gamma, eps, mm_np):
    m0 = c * ML
    Wh_l = np.asarray(Wh[m0:m0 + ML], np.float32)    # (ML, H, H) [m, h, o]
    Wx_l = np.asarray(Wx[m0:m0 + ML], np.float32)    # (ML, H, I)
    C_l = np.asarray(C[m0:m0 + ML], np.float32)      # (ML, M)
    ini = np.asarray(initial_states[m0:m0 + ML], np.float32)

    whT = Wh_l.reshape(ML, H, OC, 128).transpose(3, 0, 2, 1)  # [p, m, oc, h]
    whT = np.ascontiguousarray(whT).reshape(128, ML * OC * H).astype(mm_np)
    wmT = np.asarray(Wm, np.float32).reshape(H, OC, 128).transpose(2, 1, 0)
    wmT = np.ascontiguousarray(wmT).reshape(128, OC * H).astype(mm_np)
    cT = np.zeros((128, ML), mm_np)
    cT[:M, :] = C_l.T.astype(mm_np)
    xT = np.ascontiguousarray(np.asarray(x, np.float32).T)       # [I, T]
    wxT = np.ascontiguousarray(Wx_l.transpose(2, 0, 1)).reshape(128, ML * H)

    return {
        "whT": whT,
        "wmT": wmT,
        "cT": cT,
        "xT": xT,
        "wxT": wxT,
        "biasL": _to_L(bias[m0:m0 + ML]),
        "aL": _to_L(1.0 - DT * np.asarray(eps[m0:m0 + ML], np.float32)),
        "g2L": _to_L(DT * np.asarray(gamma[m0:m0 + ML], np.float32)),
        "hy0L": _to_L(ini[:, 0]),
        "hz0L": _to_L(ini[:, 1]),
    }


def build(nc, t_steps=T, mm_dtype=MM_DTYPE):
    import concourse.mybir as mybir
    import concourse.tile as tile
    from concourse.bass import ts
    from contextlib import ExitStack

    F32 = mybir.dt.float32
    MMDT = getattr(mybir.dt, mm_dtype)
    AF = mybir.ActivationFunctionType
    ALU = mybir.AluOpType

    d_whT = nc.dram_tensor("whT", [128, ML * OC * H], MMDT, kind="ExternalInput").ap()
    d_wmT = nc.dram_tensor("wmT", [128, OC * H], MMDT, kind="ExternalInput").ap()
    d_cT = nc.dram_tensor("cT", [128, ML], MMDT, kind="ExternalInput").ap()
    d_xT = nc.dram_tensor("xT", [128, T], F32, kind="ExternalInput").ap()
    d_wxT = nc.dram_tensor("wxT", [128, ML * H], F32, kind="ExternalInput").ap()
    d_biasL = nc.dram_tensor("biasL", [128, NCOLS], F32, kind="ExternalInput").ap()
    d_aL = nc.dram_tensor("aL", [128, NCOLS], F32, kind="ExternalInput").ap()
    d_g2L = nc.dram_tensor("g2L", [128, NCOLS], F32, kind="ExternalInput").ap()
    d_hy0 = nc.dram_tensor("hy0L", [128, NCOLS], F32, kind="ExternalInput").ap()
    d_hz0 = nc.dram_tensor("hz0L", [128, NCOLS], F32, kind="ExternalInput").ap()

    # outputs stay in canonical layout on device; host transposes
    d_states = nc.dram_tensor(
        "out_states", [t_steps, 2, 128, NCOLS], F32, kind="ExternalOutput"
    ).ap()
    d_fb = nc.dram_tensor(
        "out_fb", [t_steps, 128, NCOLS], F32, kind="ExternalOutput"
    ).ap()

    with tile.TileContext(nc) as tc, ExitStack() as ctx:
        const = ctx.enter_context(tc.tile_pool(name="const", bufs=1))

        whT = const.tile([128, ML * OC * H], MMDT)
        # per-(m,oc) chunk loads so each consumer matmul waits on one queue
        for mo in range(ML * OC):
            nc.sync.dma_start(whT[:, ts(mo, H)], d_whT[:, ts(mo, H)])
        wmT = const.tile([128, OC * H], MMDT)
        for oc in range(OC):
            nc.sync.dma_start(wmT[:, ts(oc, H)], d_wmT[:, ts(oc, H)])
        cT = const.tile([128, ML], MMDT)
        nc.sync.dma_start(cT[:], d_cT)
        biasL = const.tile([128, NCOLS], F32)
        nc.sync.dma_start(biasL[:], d_biasL)
        aL = const.tile([128, NCOLS], F32)
        nc.sync.dma_start(aL[:], d_aL)
        g2L = const.tile([128, NCOLS], F32)
        nc.sync.dma_start(g2L[:], d_g2L)
        pxb = const.tile([128, t_steps * NCOLS], F32)

        # ---- px precompute: pxb[:, t*NCOLS + hc*ML+m] = (Wx[m] @ x_t)[h] + bias ----
        with tc.tile_pool(name="pxio", bufs=1) as pxio, \
             tc.tile_pool(name="pxps", bufs=2, space="PSUM") as pxps:
            xT = pxio.tile([128, T], F32)
            nc.sync.dma_start(xT[:], d_xT)
            wxT = pxio.tile([128, ML * H], F32)
            for m in range(ML):
                nc.sync.dma_start(wxT[:, ts(m, H)], d_wxT[:, ts(m, H)])
            pxb3 = pxb[:].rearrange("p (t c) -> p t c", c=NCOLS)
            for m in range(ML):
                for hc in range(OC):
                    ps = pxps.tile([128, t_steps], F32, tag="pxps")
                    nc.tensor.matmul(
                        ps[:],
                        lhsT=wxT[:, m * H + hc * 128: m * H + hc * 128 + 128],
                        rhs=xT[:, 0:t_steps],
                        start=True, stop=True,
                    )
                    nc.vector.tensor_scalar_add(
                        pxb3[:, :, hc * ML + m], ps[:],
                        biasL[:, hc * ML + m: hc * ML + m + 1],
                    )

        psum_pre = ctx.enter_context(tc.tile_pool(name="prep", bufs=3, space="PSUM"))
        psum_fb = ctx.enter_context(tc.tile_pool(name="fbp", bufs=2, space="PSUM"))
        psum_yt = ctx.enter_context(tc.tile_pool(name="ytp", bufs=2, space="PSUM"))
        sb = ctx.enter_context(tc.tile_pool(name="sb", bufs=4))
        state = ctx.enter_context(tc.tile_pool(name="state", bufs=4))
        dram = ctx.enter_context(tc.tile_pool(name="dram", bufs=4, space="DRAM"))
        # collective outputs must be addr_space=Shared for the fast path
        ag_outs = [
            nc.dram_tensor(f"ag_out_{i}", [NCORES * NCOLS, 128], MMDT,
                           addr_space="Shared")
            for i in range(4)
        ]

        outs_sb = []
        for i in range(2):
            o = const.tile([128, H], MMDT, tag=f"outs{i}")
            nc.vector.memset(o[M:128, :], 0.0)
            outs_sb.append(o)

        hyL = state.tile([128, NCOLS], F32, tag="hy")
        nc.sync.dma_start(hyL[:], d_hy0)
        hzL = state.tile([128, NCOLS], F32, tag="hz")
        nc.sync.dma_start(hzL[:], d_hz0)

        cast = MMDT != F32

        for t in range(t_steps):
            # ---- AllGather hy_t (outs_0 is zeros, so skip at t=0) ----
            if t > 0:
                # cast/copy into (m, oc) column order so the transposed AG
                # payload rows are module-major and the gathered buffer is
                # a flat [M, H] image
                src = sb.tile([128, NCOLS], MMDT, tag="hycast")
                nc.vector.tensor_copy(
                    src[:].rearrange("p (m oc) -> p oc m", oc=OC),
                    hyL[:].rearrange("p (oc m) -> p oc m", m=ML))
                hyT = sb.tile([NCOLS, 128], MMDT, tag="hyT")
                for bp in range(128 // 32):
                    nc.vector.transpose(
                        hyT[:, ts(bp, 32)], src[ts(bp, 32), :])
                ag_in = dram.tile([NCOLS, 128], MMDT, tag="agin")
                ag_out = ag_outs[t % 4]
                nc.sync.dma_start(ag_in[:], hyT[:])
                nc.gpsimd.collective_compute(
                    "AllGather", mybir.AluOpType.bypass,
                    replica_groups=[list(range(NCORES))],
                    ins=[ag_in[:].opt()], outs=[ag_out.ap().opt()],
                )
                outs_t = outs_sb[t % 2]
                # ag_out rows are (rank, m, oc): two half-loads on separate
                # queues; each covers 2 oc-chunks so PE wakes at most twice
                ag3 = ag_out.ap().rearrange(
                    "(mg q) p -> mg q p", q=OC)
                for half in range(2):
                    eng = nc.sync if half == 0 else nc.scalar
                    o0 = half * 2
                    eng.dma_start(
                        outs_t[0:M, o0 * 128:(o0 + 2) * 128]
                        .rearrange("mg (q p) -> mg q p", p=128),
                        ag3[:, o0:o0 + 2])

            # ---- pre = Wh @ hy (psum) ; px+bias+fb added later on DVE ----
            pre_ps = psum_pre.tile([128, NCOLS], F32, tag="pre")
            # Wh matmul rhs columns: at t=0 there is no cast tile yet
            if t == 0:
                src = sb.tile([128, NCOLS], MMDT, tag="hycast")
                nc.vector.tensor_copy(
                    src[:].rearrange("p (m oc) -> p oc m", oc=OC),
                    hyL[:].rearrange("p (oc m) -> p oc m", m=ML))
            for m in range(ML):
                for hc in range(OC):
                    for oc in range(OC):
                        base = (m * OC + oc) * H + hc * 128
                        nc.tensor.matmul(
                            pre_ps[:, hc * ML + m: hc * ML + m + 1],
                            lhsT=whT[:, base: base + 128],
                            rhs=src[:, m * OC + oc: m * OC + oc + 1],
                            start=(oc == 0), stop=(oc == OC - 1),
                        )

            preX = sb.tile([128, NCOLS], F32, tag="preX")
            nc.vector.tensor_add(preX[:], pre_ps[:], pxb[:, ts(t, NCOLS)])

            # ---- feedback fb = (C_local @ outs) @ Wm.T ----
            fb_sb = sb.tile([128, NCOLS], F32, tag="fbs")
            if t == 0:
                nc.vector.memset(fb_sb[:], 0.0)
            else:
                yt_ps = psum_yt.tile([128, NCOLS], F32, tag="yt")
                yt_sb = sb.tile([128, NCOLS], MMDT, tag="yts")
                for oc in range(OC):
                    nc.tensor.matmul(
                        yt_ps[:, ts(oc, ML)],
                        lhsT=outs_t[:, ts(oc, 128)], rhs=cT[:],
                        start=True, stop=True,
                    )
                nc.vector.tensor_copy(yt_sb[:], yt_ps[:])
                fb_ps = psum_fb.tile([128, NCOLS], F32, tag="fb")
                for hc in range(OC):
                    for oc in range(OC):
                        nc.tensor.matmul(
                            fb_ps[:, ts(hc, ML)],
                            lhsT=wmT[:, oc * H + hc * 128: oc * H + hc * 128 + 128],
                            rhs=yt_sb[:, ts(oc, ML)],
                            start=(oc == 0), stop=(oc == OC - 1),
                        )

            # tanh input = (Wh@hy + px + bias) [precomputed in preX] + fb
            th_in = sb.tile([128, NCOLS], F32, tag="thin")
            if t == 0:
                nc.vector.tensor_copy(th_in[:], preX[:])
            else:
                nc.vector.tensor_add(th_in[:], fb_ps[:], preX[:])
            th = sb.tile([128, NCOLS], F32, tag="th")
            nc.scalar.activation(th[:], th_in[:], AF.Tanh)
            if t > 0:
                nc.scalar.copy(fb_sb[:], fb_ps[:])
            nc.gpsimd.dma_start(d_fb[t], fb_sb[:])

            # ---- hz' = a*hz - (DT*g)*hy + DT*tanh ; hy' = hy + DT*hz' ----
            # hz' = t3 + DT*th ; hy' = hy + DT*hz' = (hy + DT*t3) + DT^2*th
            hyn = state.tile([128, NCOLS], F32, tag="hy")
            hzn = state.tile([128, NCOLS], F32, tag="hz")
            tmp = sb.tile([128, NCOLS], F32, tag="tmp")
            tmp2 = sb.tile([128, NCOLS], F32, tag="tmp2")
            tmp4 = sb.tile([128, NCOLS], F32, tag="tmp4")
            nc.vector.tensor_mul(tmp[:], hzL[:], aL[:])
            nc.vector.tensor_mul(tmp2[:], hyL[:], g2L[:])
            nc.vector.tensor_sub(tmp[:], tmp[:], tmp2[:])
            nc.vector.scalar_tensor_tensor(
                tmp4[:], tmp[:], DT, hyL[:], op0=ALU.mult, op1=ALU.add)
            nc.vector.scalar_tensor_tensor(
                hyn[:], th[:], DT * DT, tmp4[:], op0=ALU.mult, op1=ALU.add)

            nc.vector.scalar_tensor_tensor(
                hzn[:], th[:], DT, tmp[:], op0=ALU.mult, op1=ALU.add)
            nc.gpsimd.dma_start(d_states[t, 0], hyn[:])
            nc.gpsimd.dma_start(d_states[t, 1], hzn[:])
            hyL, hzL = hyn, hzn

    return nc


_CACHE = {}


def _get_nc(t_steps=T, mm_dtype=MM_DTYPE):
    key = (t_steps, mm_dtype)
    if key not in _CACHE:
        import concourse.bacc as bacc
        nc = bacc.Bacc("TRN2", target_bir_lowering=False, debug=False,
                       num_devices=NCORES)
        build(nc, t_steps, mm_dtype)
        nc.compile()
        _CACHE[key] = nc
    return _CACHE[key]


def run(t_steps=T, mm_dtype=MM_DTYPE, trace=False, **inputs):
    import ml_dtypes
    from concourse.bass_utils import run_bass_kernel_spmd

    mm_np = np.float32 if mm_dtype == "float32" else ml_dtypes.bfloat16
    nc = _get_nc(t_steps, mm_dtype)
    in_maps = [
        _prep_core_inputs(c, mm_np=mm_np, **inputs) for c in range(NCORES)
    ]
    res = run_bass_kernel_spmd(
        nc, in_maps, core_ids=list(range(NCORES)), trace=trace,
    )

    initial_states = np.asarray(inputs["initial_states"], np.float32)
    states_all = np.empty((t_steps + 1, M, 2, H), np.float32)
    states_all[0] = initial_states
    seq_fb = np.empty((t_steps, M, H), np.float32)
    for c in range(NCORES):
        r = res.results[c]
        # out_states [t, 2, 128, NCOLS] canonical -> (t, ML, 2, H)
        st = r["out_states"].reshape(t_steps, 2, 128, OC, ML)
        states_all[1:, c * ML:(c + 1) * ML] = st.transpose(0, 4, 1, 3, 2).reshape(
            t_steps, ML, 2, H)
        fb = r["out_fb"].reshape(t_steps, 128, OC, ML)
        seq_fb[:, c * ML:(c + 1) * ML] = fb.transpose(0, 3, 2, 1).reshape(
            t_steps, ML, H)
    return (states_all, seq_fb), res


def kernel(**inputs):
    (states_all, seq_fb), _ = run(**inputs)
    return states_all, seq_fb
